# revision 27
# baseline (speedup 1.0000x reference)
"""Batch multi-head graph attention (GAT) kernel for 8 Trainium2 NeuronCores.

Reference computation (per batch b, head g):
    hp   = h[b] @ w[g]                        # [N, O]
    t    = tanh(hp)
    src  = t @ a_src[g];  dst = t @ a_dst[g]  # [N]
    s    = leaky_relu(src[q] + dst[m], 0.2)   # [N(q), N(m)]
    e    = exp(s) masked by adj[b][q, m]
    out  = (e @ hp) / rowsum(e) + bias

Device strategy (per core; core c -> b = c//2, heads = (2*(c%2), 2*(c%2)+1)):
  * scores are built TRANSPOSED: tiles [128 m(keys), 2048 q(queries)] so the
    output matmul out^T[o, q] = sum_m hp[m, o] * e[m, q] streams on PE with
    the contraction dim on partitions.  The adjacency mask is pre-transposed
    on the host and streamed in via a dtype-casting DMA (u8 -> bf16).
  * exp factorization with query-scale cancellation:
        exp(leaky(s)) = max(exp(s), exp(0.2 s))
                      = exp(0.2 src)[q] * max(exp(0.8 src)[q] * exp(dst)[m],
                                              exp(0.2 dst)[m])
    Both the output numerator and the softmax denominator are accumulated by
    the same matmul, so the common per-query factor exp(0.2 src)[q] cancels
    in the final division and is never computed.  Per m-chunk the scores are
    ONE dual-op tensor_scalar (DVE 4x perf mode):
        u = (c_b * P[m]) max Q[m]      c_b = exp(0.8 src) broadcast tile
    plus one mask multiply per adjacency slab.
  * B2 rides PE/ACT instead of DVE: per 2-chunk group, PE transposes the
    tanh block [128 m, 128 (c,o)] -> [128 (c,o), 128 m] and a 4-column
    matvec against (a_src|a_dst) yields sd[m, (src,dst) x 2 chunks] directly
    partition-oriented.  exp(dst)/exp(0.2 dst) come off sd with strided ACT
    exps; exp(0.8 src) is broadcast via the sel-matmul trick.  The DVE does
    nothing in B1/B2 except the hp PSUM->SBUF copies.
  * softmax denominator rides as a ones-column in the matmul lhsT; bias is
    folded into hp (softmax weights sum to exactly 1).  Final transpose back
    to [q, o] via PE transpose; normalization runs on the scalar engine.
  * PSUM banks (8): B1/B2/B4 transient tiles share a 3-bank rotation
    ("work"), sd gets 1 persistent bank, the output accumulator takes 4.
"""

import math
import os
import sys

for _p in ("/opt/trn_rl_repo",):
    if _p not in sys.path and os.path.isdir(_p):
        sys.path.insert(0, _p)

from contextlib import ExitStack

import numpy as np

import concourse.bass as bass
import concourse.tile as tile
from concourse import bacc, mybir
from concourse.bass_utils import run_bass_kernel_spmd
from concourse.tile_rust import add_dep_helper

F32 = mybir.dt.float32
BF16 = mybir.dt.float16  # fp16: more mantissa + possible DVE fast-path
U8 = mybir.dt.uint8
F8 = mybir.dt.float8e4
AF = mybir.ActivationFunctionType
OP = mybir.AluOpType
AX = mybir.AxisListType

N = 2048          # nodes
F = 256           # input features
O = 64            # output features
W = O + 1         # hp columns + ones column
P = 128           # partitions
NCH = N // P      # 16 m-chunks per pair
NGR = NCH // 2    # 2-chunk B1 groups
NEG_SLOPE = 0.2
N_CORES = 8
NADJ = 8          # adj arrives as NADJ independent slabs for overlap
MPER = NCH // NADJ

# masked scores written as fp8e4 (u <= 187/16 after folding 1/16 into the exp
# biases; the scale cancels in the softmax division). Halves the mask op's
# write bytes (the mask tensor_tensor is SBUF byte-bandwidth-bound on HW).
U_FP8 = os.environ.get("KERNEL_U_FP8", "0") == "1"
U_SCALE_BIAS = -math.log(16.0)  # exp bias: scales cb and Q by 1/16
# when the bias input is all-zero (it is for this problem), hp needs no add;
# set per-build from the actual bias values in kernel()
ZERO_BIAS = True


class PairCtx:
    pass


def _emit_b1b2(nc, cpool, pspool, consts, hT, pair):
    """projections + score ingredients for one head (pair index).

    Per 2-chunk group: hp matmuls into a [128, 128] PSUM group tile, one
    copy to hp_big (bf16, strided over the ones column), one ACT tanh, a PE
    transpose of the tanh block, an ACT copy of the transpose to SBUF, and a
    4-col PE matvec accumulating sd[m, (src,dst)] per chunk.

    pair 0's small copies ride the DVE (idle during B1); pair 1's ride ACT
    so they never sit in the DVE queue ahead of pair 0's mask stream."""
    ident_sb, sel_sb, w_sb, a4_sb, bias_sb, nbias_sb = consts

    def copy(dst, src):
        if pair == 0:
            nc.vector.tensor_copy(dst, src)
        else:
            nc.scalar.copy(dst, src)
    px = PairCtx()
    px.hp_big = cpool.tile([P, NCH * W], BF16, tag="hp_big", bufs=2, name=f"hp{pair}")
    px.t_cat = cpool.tile([P, NCH * O], F32, tag="t_cat", bufs=2, name=f"tcat{pair}")

    sd_ps = pspool.tile([P, 4 * NGR], F32, tag="sd", bufs=1, name=f"sd{pair}")

    def emit_group_head(g):
        psum_hp = pspool.tile([P, 2 * O], F32, tag="work", bufs=3,
                              name=f"pshp{pair}_g{g}")
        for k in range(2):
            mc = 2 * g + k
            for fc in range(2):
                nc.tensor.matmul(
                    psum_hp[:, k * O:(k + 1) * O],
                    hT[fc][:, mc * P:(mc + 1) * P],
                    w_sb[:, (2 * pair + fc) * O:(2 * pair + fc + 1) * O],
                    start=(fc == 0),
                    stop=(fc == 1),
                )
        if ZERO_BIAS:
            copy(
                px.hp_big.rearrange("p (c k) -> p c k", k=W)[:, 2 * g:2 * g + 2, 0:O],
                psum_hp.rearrange("p (c k) -> p c k", k=O),
            )
        else:
            for k in range(2):
                mc = 2 * g + k
                nc.vector.tensor_tensor(
                    px.hp_big[:, mc * W:mc * W + O],
                    psum_hp[:, k * O:(k + 1) * O], bias_sb[:], OP.add,
                )
        nc.scalar.activation(
            px.t_cat[:, 2 * g * O:(2 * g + 2) * O], psum_hp[:], AF.Tanh
        )
        return psum_hp

    def emit_group_tail(g):
        # transpose the tanh block: [128 m, 128 (c,o)] -> [128 (c,o), 128 m]
        tT2_ps = pspool.tile([P, P], F32, tag="work", bufs=3,
                             name=f"tT2ps{pair}_{g}")
        nc.tensor.transpose(
            tT2_ps[:], px.t_cat[:, 2 * g * O:(2 * g + 2) * O], ident_sb[:]
        )
        tT2_sb = cpool.tile([P, P], F32, tag="tT2", bufs=3, name=f"tT2{pair}_{g}")
        copy(tT2_sb[:], tT2_ps[:])
        # matvec: out[m, (src_e, dst_e, src_o, dst_o)] for the group's chunks
        nc.tensor.matmul(
            sd_ps[:, 4 * g:4 * g + 4],
            tT2_sb[:],
            a4_sb[:, 4 * pair:4 * pair + 4],
            start=True, stop=True,
        )

    # software-pipelined emission: group g's PE tail follows group g+1's
    # matmuls so the in-order PE queue never stalls on ACT's tanh.
    for g in range(NGR):
        emit_group_head(g)
        if g >= 1:
            emit_group_tail(g - 1)
    emit_group_tail(NGR - 1)

    # ---- B2 tail: exp vectors + src broadcast ----
    ones_cols = px.hp_big.rearrange("p (c k) -> p c k", k=W)[:, :, O:O + 1]
    (nc.vector if pair == 0 else nc.gpsimd).memset(ones_cols, 1.0)

    sd_sb = cpool.tile([P, 4 * NGR], F32, tag="sd_sb", bufs=2, name=f"sdsb{pair}")
    copy(sd_sb[:], sd_ps[:])
    sd3 = sd_sb.rearrange("p (c k) -> p c k", k=2)
    px.edst = cpool.tile([P, NCH], F32, tag="edst", bufs=2, name=f"edst{pair}")
    px.edst02 = cpool.tile([P, NCH], F32, tag="edst02", bufs=2, name=f"edst02{pair}")
    nc.scalar.activation(px.edst[:], sd3[:, :, 1], AF.Exp)
    nc.scalar.activation(px.edst02[:], sd3[:, :, 1], AF.Exp, scale=NEG_SLOPE,
                         bias=nbias_sb[:] if U_FP8 else 0.0)

    # build c_b = exp(0.8 src)[q] broadcast over partitions with no DMA:
    # PE-transpose the src columns of sd into q-major rows, then 16 rank-1
    # (sel x row) matmuls into PSUM; exp runs on the way out of PSUM.
    ps_srcT = pspool.tile([NCH, P], F32, tag="work", bufs=3, name=f"srcT{pair}")
    nc.tensor.transpose(ps_srcT[:], sd3[:, :, 0], ident_sb[:])
    srowT = cpool.tile([NCH, P], F32, tag="srowT", bufs=2, name=f"srowT{pair}")
    copy(srowT[:], ps_srcT[:])
    px.c_b = cpool.tile([P, N], BF16, tag="c_b", bufs=2, name=f"cb{pair}")
    for piece in range(4):
        ps_bc = pspool.tile([P, 512], F32, tag="work", bufs=3,
                            name=f"psbc{pair}_{piece}")
        for c4 in range(4):
            c = piece * 4 + c4
            nc.tensor.matmul(
                ps_bc[:, c4 * P:(c4 + 1) * P], sel_sb[:, c * P:(c + 1) * P],
                srowT[:], start=True, stop=True,
            )
        nc.scalar.activation(
            px.c_b[:, piece * 512:(piece + 1) * 512], ps_bc[:], AF.Exp,
            scale=1.0 - NEG_SLOPE,
            bias=nbias_sb[:] if U_FP8 else 0.0,
        )
    return px


def _emit_b3(nc, epool, pspool, adj_slabs, pair, px):
    """scores + output matmul accumulation over m-chunks."""
    psum_out = pspool.tile([W, N], F32, tag="big", bufs=1, name=f"psout{pair}")
    us = {}  # mc -> (tile, element offset of the chunk within the tile)
    # one mask tensor_tensor per adjacency slab (MPER chunks): halves the
    # per-op overhead + semaphore traffic on the DVE's critical stream
    for slab in range(NADJ):
        v2 = epool.tile([P, MPER * N], BF16, tag="v2", bufs=2,
                        name=f"v2_{pair}_{slab}")
        u2 = epool.tile([P, MPER * N], F8 if U_FP8 else BF16, tag="u2",
                        bufs=5, name=f"u2_{pair}_{slab}")
        if (pair == 0 and slab == 0) or slab == NADJ - 1:
            # split this slab's score ops per 512-col q piece.  For the first
            # slab the first tensor_scalar then starts as soon as c_b's first
            # piece lands instead of waiting for the full broadcast build;
            # for the last slab the output matmuls overlap the masking so the
            # B4 tail starts ~2us earlier.
            for k in range(MPER):
                mc = slab * MPER + k
                for j in range(4):
                    q0, q1 = j * 512, (j + 1) * 512
                    nc.vector.tensor_scalar(
                        v2[:, k * N + q0:k * N + q1], px.c_b[:, q0:q1],
                        px.edst[:, mc:mc + 1], px.edst02[:, mc:mc + 1],
                        OP.mult, OP.max,
                    )
                us[mc] = (u2, k * N)
            for j in range(4):
                nc.vector.tensor_tensor(
                    u2.rearrange("p (k q) -> p k q", q=N)[:, :, j * 512:(j + 1) * 512],
                    v2.rearrange("p (k q) -> p k q", q=N)[:, :, j * 512:(j + 1) * 512],
                    adj_slabs[slab].rearrange("p (k q) -> p k q", q=N)[:, :, j * 512:(j + 1) * 512],
                    OP.mult,
                )
            continue
        for k in range(MPER):
            mc = slab * MPER + k
            nc.vector.tensor_scalar(
                v2[:, k * N:(k + 1) * N], px.c_b[:],
                px.edst[:, mc:mc + 1], px.edst02[:, mc:mc + 1],
                OP.mult, OP.max,
            )
            us[mc] = (u2, k * N)
        nc.vector.tensor_tensor(u2[:], v2[:], adj_slabs[slab][:], OP.mult)
    def emit_mm(mc, j):
        u_tile, uoff = us[mc]
        nc.tensor.matmul(
            psum_out[:, j * 512:(j + 1) * 512],
            px.hp_big[:, mc * W:(mc + 1) * W],
            u_tile[:, uoff + j * 512:uoff + (j + 1) * 512],
            start=(mc == 0),
            stop=(mc == NCH - 1),
            skip_group_check=True,
        )

    for mc in range(NCH - MPER):
        for j in range(4):
            emit_mm(mc, j)
    # last slab: j-major so each piece's matmuls hide behind the next
    # piece's mask op instead of queueing behind the final one
    for j in range(4):
        for mc in range(NCH - MPER, NCH):
            emit_mm(mc, j)
    return psum_out


def _emit_b4(nc, cpool, pspool, ident_sb, pair, psum_out, out_d):
    """transpose back, normalize, store.  Pipelined by 512-col q pieces: the
    PSUM->SBUF copy (needed because PE can't read PSUM) runs per piece on
    ACT; the per-chunk reciprocal and scale read the transposed PSUM tile
    directly, so nothing round-trips through SBUF.

    pair 0's B4 overlaps pair 1's DVE-bound B3: its recips ride gpsimd and
    scales ride ACT so the DVE mask stream is never blocked.  pair 1's B4 is
    the kernel tail where the DVE is idle: recips + scales ride the DVE."""
    outT_sb = cpool.tile([W, N], F32, tag="outT", bufs=1, name=f"outT{pair}")
    outR = (cpool.tile([P, NCH * W], F32, tag="outR", bufs=1, name=f"outR{pair}")
            if pair == 0 else None)
    rec = cpool.tile([P, NCH], F32, tag="rec", bufs=2, name=f"rec{pair}")
    out_sb = cpool.tile([P, NCH * O], F32, tag="out_sb", bufs=1, name=f"outsb{pair}")
    GRP = 4
    for qg in range(NCH // GRP):
        # pair 1's tail: the DVE is idle, so split the piece copies and the
        # normalization across ACT+DVE and Pool+DVE respectively
        if pair == 1 and qg % 2 == 1:
            nc.vector.tensor_copy(outT_sb[:, qg * 512:(qg + 1) * 512],
                                  psum_out[:, qg * 512:(qg + 1) * 512])
        else:
            nc.scalar.copy(outT_sb[:, qg * 512:(qg + 1) * 512],
                           psum_out[:, qg * 512:(qg + 1) * 512])
        for qc in range(qg * GRP, (qg + 1) * GRP):
            psum_t = pspool.tile(
                [P, W], F32, tag="work", bufs=3, name=f"pst{pair}_{qc}"
            )
            nc.tensor.transpose(
                psum_t[:], outT_sb[:, qc * P:(qc + 1) * P], ident_sb[:W, :W]
            )
            # normalize by the ones-column rowsum.  Only the DVE can
            # reciprocal; pair 1 (kernel tail, DVE idle) runs recip+mul per
            # chunk straight from PSUM, pair 0 stages through SBUF on ACT
            # with one batched DVE recip per group so the DVE mask stream
            # is barely touched.
            if pair == 1:
                nc.vector.reciprocal(rec[:, qc:qc + 1], psum_t[:, O:O + 1])
                nc.vector.tensor_scalar_mul(
                    out_sb[:, qc * O:(qc + 1) * O], psum_t[:, 0:O],
                    rec[:, qc:qc + 1],
                )
            else:
                nc.scalar.copy(outR[:, qc * W:(qc + 1) * W], psum_t[:])
        if pair == 0:
            den = outR.rearrange("p (c k) -> p c k", k=W)[:, qg * GRP:(qg + 1) * GRP, O:O + 1]
            nc.vector.reciprocal(
                rec.rearrange("p (c k) -> p c k", k=1)[:, qg * GRP:(qg + 1) * GRP, :], den
            )
            for qc in range(qg * GRP, (qg + 1) * GRP):
                nc.scalar.activation(
                    out_sb[:, qc * O:(qc + 1) * O], outR[:, qc * W:qc * W + O],
                    AF.Copy, scale=rec[:, qc:qc + 1],
                )
        nc.sync.dma_start(
            out_d[pair].rearrange("(c p) o -> p c o", p=P)[:, qg * GRP:(qg + 1) * GRP, :],
            out_sb.rearrange("p (c k) -> p c k", k=O)[:, qg * GRP:(qg + 1) * GRP, :],
        )


def build_program(reps=1, loop_trip=None):
    nc = bacc.Bacc(
        "TRN2",
        target_bir_lowering=False,
        debug=False,
        enable_asserts=True,
        num_devices=1,
    )
    ht_d = nc.dram_tensor("ht", [F, N], F32, kind="ExternalInput").ap()
    adjt_d = nc.dram_tensor("adjt", [N, N], U8, kind="ExternalInput").ap()
    w_d = nc.dram_tensor("w", [2, F, O], F32, kind="ExternalInput").ap()
    a4_d = nc.dram_tensor("a4", [P, 8], F32, kind="ExternalInput").ap()
    biasb_d = nc.dram_tensor("biasb", [P, O], F32, kind="ExternalInput").ap()
    ident_d = nc.dram_tensor("ident", [P, P], F32, kind="ExternalInput").ap()
    sel_d = nc.dram_tensor("sel", [NCH, N], F32, kind="ExternalInput").ap()
    out_d = nc.dram_tensor("out", [2, N, O], F32, kind="ExternalOutput").ap()

    with tile.TileContext(nc) as tc, ExitStack() as ctx:
        consts_pool = ctx.enter_context(tc.tile_pool(name="consts", bufs=1))
        hpool = ctx.enter_context(tc.tile_pool(name="hpool", bufs=1))
        cpool = ctx.enter_context(tc.tile_pool(name="cpool", bufs=1))
        epool = ctx.enter_context(tc.tile_pool(name="epool", bufs=1))
        pspool = ctx.enter_context(tc.tile_pool(name="psum", bufs=1, space="PSUM"))

        # ACT func table preload: a dummy activation depending only on a
        # memset pulls the 1283ns LoadActFuncSet off the first tanh's
        # critical path.
        nbias_sb = consts_pool.tile([P, 1], F32, tag="nbias")
        nc.vector.memset(nbias_sb[:], U_SCALE_BIAS)
        actwarm = consts_pool.tile([1, 1], F32, tag="actwarm")
        nc.scalar.activation(actwarm[:], nbias_sb[0:1, 0:1], AF.Tanh)

        # --- priority DMAs, spread across three HWDGE queues (each queue
        # dispatches serially at ~625ns/DMA): w + h fc0 on sync, a4/sel +
        # h fc1 on scalar, ident/bias on vector.
        w_sb = consts_pool.tile([P, 2 * 2 * O], F32, tag="w")
        nc.sync.dma_start(
            w_sb.rearrange("k (h c o) -> k h c o", h=2, c=2),
            w_d.rearrange("h (c k) o -> k h c o", k=P),
        )
        a4_sb = consts_pool.tile([P, 8], F32, tag="a4")
        nc.scalar.dma_start(a4_sb[:], a4_d[:])
        bias_sb = consts_pool.tile([P, O], F32, tag="bias")
        nc.gpsimd.dma_start(bias_sb[:], biasb_d[:])

        hT = [
            hpool.tile([P, N], F32, tag=f"hT{fc}", name=f"hT{fc}")
            for fc in range(2)
        ]
        h_queues = (nc.sync, nc.scalar)
        h_dmas = [None] * 8
        ident_sb = consts_pool.tile([P, P], F32, tag="ident")
        for piece in range(4):
            for fc in range(2):
                h_dmas[2 * piece + fc] = h_queues[fc].dma_start(
                    hT[fc][:, piece * 512:(piece + 1) * 512],
                    ht_d[fc * P:(fc + 1) * P, piece * 512:(piece + 1) * 512],
                )
            if piece == 1:
                # ident is first needed by the g0 transpose (~tanh(0) time);
                # don't let it delay the first two h pieces.
                nc.scalar.dma_start(ident_sb[:], ident_d[:])
        sel_sb = consts_pool.tile([NCH, N], F32, tag="sel")
        nc.gpsimd.dma_start(sel_sb[:], sel_d[:])

        if loop_trip is not None:
            from concourse.engine_type import EngineType
            _loop_cm = tc.For_i(
                0, loop_trip, 1,
                hint_engines=(EngineType.PE, EngineType.DVE,
                              EngineType.Activation, EngineType.SP,
                              EngineType.Pool),
            )
            _loop_cm.__enter__()
        for rep in range(reps):
          # adjacency: NADJ independent slabs, fp16 via cast DMA (cast DMAs
          # must initiate from gpsimd)
          adj_slabs = [
              hpool.tile([P, MPER * N], BF16, tag=f"adj{s}", name=f"adj{s}")
              for s in range(NADJ)
          ]
          adjt_r = adjt_d.rearrange("(c p) q -> p c q", p=P)  # [128, 16, 2048]
          for s in range(NADJ):
              adj_dma = nc.gpsimd.dma_start(
                  adj_slabs[s].rearrange("p (c q) -> p c q", q=N),
                  adjt_r[:, s * MPER:(s + 1) * MPER, :],
              )
              # don't compete with the latency-critical h loads for DMA
              # bandwidth; the first B3 use of adj is ~10us in.
              add_dep_helper(adj_dma.ins, h_dmas[-1].ins,
                             reason="delay adj behind h")

          consts = (ident_sb, sel_sb, w_sb, a4_sb, bias_sb, nbias_sb)
          px = [_emit_b1b2(nc, cpool, pspool, consts, hT, pair)
                for pair in range(2)]
          for pair in range(2):
              psum_out = _emit_b3(nc, epool, pspool, adj_slabs, pair, px[pair])
              _emit_b4(nc, cpool, pspool, ident_sb, pair, psum_out, out_d)

        if loop_trip is not None:
            _loop_cm.__exit__(None, None, None)

    nc.compile()
    return nc


_CACHED = {}


def _get_program(zero_bias=True):
    global ZERO_BIAS
    key = ("nc", zero_bias)
    if key not in _CACHED:
        ZERO_BIAS = zero_bias
        _CACHED[key] = build_program()
    return _CACHED[key]


def make_in_maps(h, adj, w, a_src, a_dst, bias):
    h = np.ascontiguousarray(np.asarray(h, dtype=np.float32))
    adj = np.asarray(adj)
    w = np.asarray(w, dtype=np.float32)
    a_src = np.asarray(a_src, dtype=np.float32).reshape(4, O)
    a_dst = np.asarray(a_dst, dtype=np.float32).reshape(4, O)
    bias = np.asarray(bias, dtype=np.float32).reshape(O)

    adjT = np.ascontiguousarray(adj.transpose(0, 2, 1)).astype(np.uint8)
    biasb = np.ascontiguousarray(np.broadcast_to(bias, (P, O)))
    ident = np.eye(P, dtype=np.float32)
    sel = np.kron(np.eye(NCH, dtype=np.float32), np.ones((1, P), np.float32))

    in_maps = []
    for c in range(N_CORES):
        b = c // 2
        hs = [2 * (c % 2), 2 * (c % 2) + 1]
        # a4[:, 4*pair + (0..3)]: rows 0-63 (even chunk o's) carry
        # (a_src, a_dst, 0, 0); rows 64-127 (odd chunk o's) (0, 0, a_src,
        # a_dst) for that pair's head.
        a4 = np.zeros((P, 8), np.float32)
        for pair, hd in enumerate(hs):
            a4[0:O, 4 * pair + 0] = a_src[hd]
            a4[0:O, 4 * pair + 1] = a_dst[hd]
            a4[O:P, 4 * pair + 2] = a_src[hd]
            a4[O:P, 4 * pair + 3] = a_dst[hd]
        in_maps.append({
            "ht": np.ascontiguousarray(h[b].T),
            "adjt": adjT[b],
            "w": np.ascontiguousarray(w[hs]),
            "a4": a4,
            "biasb": biasb,
            "ident": ident,
            "sel": sel,
        })
    return in_maps


def assemble(results):
    out = np.empty((4, 4, N, O), dtype=np.float32)
    for c in range(N_CORES):
        b = c // 2
        for i, hd in enumerate((2 * (c % 2), 2 * (c % 2) + 1)):
            out[b, hd] = results[c]["out"][i]
    return out


def kernel(h, adj, w, a_src, a_dst, bias):
    nc = _get_program(zero_bias=not np.any(np.asarray(bias)))
    in_maps = make_in_maps(h, adj, w, a_src, a_dst, bias)
    res = run_bass_kernel_spmd(nc, in_maps, core_ids=list(range(N_CORES)))
    return assemble(res.results)


# revision 29
# speedup vs baseline: 1.1622x; 1.1622x over previous
"""Batch multi-head graph attention (GAT) kernel for 8 Trainium2 NeuronCores.

Reference computation (per batch b, head g):
    hp   = h[b] @ w[g]                        # [N, O]
    t    = tanh(hp)
    src  = t @ a_src[g];  dst = t @ a_dst[g]  # [N]
    s    = leaky_relu(src[q] + dst[m], 0.2)   # [N(q), N(m)]
    e    = exp(s) masked by adj[b][q, m]
    out  = (e @ hp) / rowsum(e) + bias

Device strategy (per core; core c -> b = c//2, heads = (2*(c%2), 2*(c%2)+1)):
  * scores are built TRANSPOSED: tiles [128 m(keys), 2048 q(queries)] so the
    output matmul out^T[o, q] = sum_m hp[m, o] * e[m, q] streams on PE with
    the contraction dim on partitions.  The adjacency mask is pre-transposed
    on the host and streamed in via a dtype-casting DMA (u8 -> bf16).
  * exp factorization with query-scale cancellation:
        exp(leaky(s)) = max(exp(s), exp(0.2 s))
                      = exp(0.2 src)[q] * max(exp(0.8 src)[q] * exp(dst)[m],
                                              exp(0.2 dst)[m])
    Both the output numerator and the softmax denominator are accumulated by
    the same matmul, so the common per-query factor exp(0.2 src)[q] cancels
    in the final division and is never computed.  Per m-chunk the scores are
    ONE dual-op tensor_scalar (DVE 4x perf mode):
        u = (c_b * P[m]) max Q[m]      c_b = exp(0.8 src) broadcast tile
    plus one mask multiply per adjacency slab.
  * B2 rides PE/ACT instead of DVE: per 2-chunk group, PE transposes the
    tanh block [128 m, 128 (c,o)] -> [128 (c,o), 128 m] and a 4-column
    matvec against (a_src|a_dst) yields sd[m, (src,dst) x 2 chunks] directly
    partition-oriented.  exp(dst)/exp(0.2 dst) come off sd with strided ACT
    exps; exp(0.8 src) is broadcast via the sel-matmul trick.  The DVE does
    nothing in B1/B2 except the hp PSUM->SBUF copies.
  * softmax denominator rides as a ones-column in the matmul lhsT; bias is
    folded into hp (softmax weights sum to exactly 1).  Final transpose back
    to [q, o] via PE transpose; normalization runs on the scalar engine.
  * PSUM banks (8): B1/B2/B4 transient tiles share a 3-bank rotation
    ("work"), sd gets 1 persistent bank, the output accumulator takes 4.
"""

import math
import os
import sys

for _p in ("/opt/trn_rl_repo",):
    if _p not in sys.path and os.path.isdir(_p):
        sys.path.insert(0, _p)

from contextlib import ExitStack

import numpy as np

import concourse.bass as bass
import concourse.tile as tile
from concourse import bacc, mybir
from concourse.bass_utils import run_bass_kernel_spmd
from concourse.tile_rust import add_dep_helper

F32 = mybir.dt.float32
BF16 = mybir.dt.float16  # fp16: more mantissa + possible DVE fast-path
U8 = mybir.dt.uint8
F8 = mybir.dt.float8e4
AF = mybir.ActivationFunctionType
OP = mybir.AluOpType
AX = mybir.AxisListType

N = 2048          # nodes
F = 256           # input features
O = 64            # output features
W = O + 1         # hp columns + ones column
P = 128           # partitions
NCH = N // P      # 16 m-chunks per pair
NGR = NCH // 2    # 2-chunk B1 groups
NEG_SLOPE = 0.2
N_CORES = 8
NADJ = 8          # adj arrives as NADJ independent slabs for overlap
MPER = NCH // NADJ

# masked scores written as fp8e4.  REJECTED on HW: 150.6us vs 104.2us -- the
# 1-byte output drops the mask tensor_tensor off the DVE's 2-byte fast path
# (HW rel err 1.55e-2 would have passed the 2e-2 gate, but the perf loss is
# decisive).  Keep False.
U_FP8 = os.environ.get("KERNEL_U_FP8", "0") == "1"
# offload this slab index of each pair's mask to the Pool engine (fp16 mult,
# proven op): its tensor_scalars emit at the head of the pair's B3 so Pool
# can start right after the adj dispatches drain; its matmuls emit last so a
# slow Pool never blocks the accumulation.  -1 disables.
GP_SLAB = int(os.environ.get("KERNEL_GP_SLAB", "-1"))
U_SCALE_BIAS = -math.log(16.0)  # exp bias: scales cb and Q by 1/16
# when the bias input is all-zero (it is for this problem), hp needs no add;
# set per-build from the actual bias values in kernel()
ZERO_BIAS = True


class PairCtx:
    pass


def _emit_b1b2(nc, cpool, pspool, consts, hT, pair):
    """projections + score ingredients for one head (pair index).

    Per 2-chunk group: hp matmuls into a [128, 128] PSUM group tile, one
    copy to hp_big (bf16, strided over the ones column), one ACT tanh, a PE
    transpose of the tanh block, an ACT copy of the transpose to SBUF, and a
    4-col PE matvec accumulating sd[m, (src,dst)] per chunk.

    pair 0's small copies ride the DVE (idle during B1); pair 1's ride ACT
    so they never sit in the DVE queue ahead of pair 0's mask stream."""
    ident_sb, sel_sb, w_sb, a4_sb, bias_sb, nbias_sb = consts

    def copy(dst, src):
        if pair == 0:
            nc.vector.tensor_copy(dst, src)
        else:
            nc.scalar.copy(dst, src)
    px = PairCtx()
    px.hp_big = cpool.tile([P, NCH * W], BF16, tag="hp_big", bufs=2, name=f"hp{pair}")
    px.t_cat = cpool.tile([P, NCH * O], F32, tag="t_cat", bufs=2, name=f"tcat{pair}")

    sd_ps = pspool.tile([P, 4 * NGR], F32, tag="sd", bufs=1, name=f"sd{pair}")

    def emit_group_head(g):
        psum_hp = pspool.tile([P, 2 * O], F32, tag="work", bufs=3,
                              name=f"pshp{pair}_g{g}")
        for k in range(2):
            mc = 2 * g + k
            for fc in range(2):
                nc.tensor.matmul(
                    psum_hp[:, k * O:(k + 1) * O],
                    hT[fc][:, mc * P:(mc + 1) * P],
                    w_sb[:, (2 * pair + fc) * O:(2 * pair + fc + 1) * O],
                    start=(fc == 0),
                    stop=(fc == 1),
                )
        if ZERO_BIAS:
            copy(
                px.hp_big.rearrange("p (c k) -> p c k", k=W)[:, 2 * g:2 * g + 2, 0:O],
                psum_hp.rearrange("p (c k) -> p c k", k=O),
            )
        else:
            for k in range(2):
                mc = 2 * g + k
                nc.vector.tensor_tensor(
                    px.hp_big[:, mc * W:mc * W + O],
                    psum_hp[:, k * O:(k + 1) * O], bias_sb[:], OP.add,
                )
        nc.scalar.activation(
            px.t_cat[:, 2 * g * O:(2 * g + 2) * O], psum_hp[:], AF.Tanh
        )
        return psum_hp

    def emit_group_tail(g):
        # transpose the tanh block: [128 m, 128 (c,o)] -> [128 (c,o), 128 m]
        tT2_ps = pspool.tile([P, P], F32, tag="work", bufs=3,
                             name=f"tT2ps{pair}_{g}")
        nc.tensor.transpose(
            tT2_ps[:], px.t_cat[:, 2 * g * O:(2 * g + 2) * O], ident_sb[:]
        )
        tT2_sb = cpool.tile([P, P], F32, tag="tT2", bufs=3, name=f"tT2{pair}_{g}")
        copy(tT2_sb[:], tT2_ps[:])
        # matvec: out[m, (src_e, dst_e, src_o, dst_o)] for the group's chunks
        nc.tensor.matmul(
            sd_ps[:, 4 * g:4 * g + 4],
            tT2_sb[:],
            a4_sb[:, 4 * pair:4 * pair + 4],
            start=True, stop=True,
        )

    # software-pipelined emission: group g's PE tail follows group g+1's
    # matmuls so the in-order PE queue never stalls on ACT's tanh.
    for g in range(NGR):
        emit_group_head(g)
        if g >= 1:
            emit_group_tail(g - 1)
    emit_group_tail(NGR - 1)

    # ---- B2 tail: exp vectors + src broadcast ----
    ones_cols = px.hp_big.rearrange("p (c k) -> p c k", k=W)[:, :, O:O + 1]
    (nc.vector if pair == 0 else nc.gpsimd).memset(ones_cols, 1.0)

    sd_sb = cpool.tile([P, 4 * NGR], F32, tag="sd_sb", bufs=2, name=f"sdsb{pair}")
    copy(sd_sb[:], sd_ps[:])
    sd3 = sd_sb.rearrange("p (c k) -> p c k", k=2)
    px.edst = cpool.tile([P, NCH], F32, tag="edst", bufs=2, name=f"edst{pair}")
    px.edst02 = cpool.tile([P, NCH], F32, tag="edst02", bufs=2, name=f"edst02{pair}")
    nc.scalar.activation(px.edst[:], sd3[:, :, 1], AF.Exp)
    nc.scalar.activation(px.edst02[:], sd3[:, :, 1], AF.Exp, scale=NEG_SLOPE,
                         bias=nbias_sb[:] if U_FP8 else 0.0)

    # build c_b = exp(0.8 src)[q] broadcast over partitions with no DMA:
    # PE-transpose the src columns of sd into q-major rows, then 16 rank-1
    # (sel x row) matmuls into PSUM; exp runs on the way out of PSUM.
    ps_srcT = pspool.tile([NCH, P], F32, tag="work", bufs=3, name=f"srcT{pair}")
    nc.tensor.transpose(ps_srcT[:], sd3[:, :, 0], ident_sb[:])
    srowT = cpool.tile([NCH, P], F32, tag="srowT", bufs=2, name=f"srowT{pair}")
    copy(srowT[:], ps_srcT[:])
    px.c_b = cpool.tile([P, N], BF16, tag="c_b", bufs=2, name=f"cb{pair}")
    for piece in range(4):
        ps_bc = pspool.tile([P, 512], F32, tag="work", bufs=3,
                            name=f"psbc{pair}_{piece}")
        for c4 in range(4):
            c = piece * 4 + c4
            nc.tensor.matmul(
                ps_bc[:, c4 * P:(c4 + 1) * P], sel_sb[:, c * P:(c + 1) * P],
                srowT[:], start=True, stop=True,
            )
        nc.scalar.activation(
            px.c_b[:, piece * 512:(piece + 1) * 512], ps_bc[:], AF.Exp,
            scale=1.0 - NEG_SLOPE,
            bias=nbias_sb[:] if U_FP8 else 0.0,
        )
    return px


def _emit_b3(nc, epool, pspool, adj_slabs, pair, px):
    """scores + output matmul accumulation over m-chunks."""
    psum_out = pspool.tile([W, N], F32, tag="big", bufs=1, name=f"psout{pair}")
    us = {}  # mc -> (tile, element offset of the chunk within the tile)

    def emit_slab(slab, v2, u2, split, eng):
        """one mask tensor_tensor per adjacency slab (MPER chunks): halves
        the per-op overhead + semaphore traffic on the DVE's critical
        stream.  split=True splits the score ops per 512-col q piece (first
        slab: start as soon as c_b's first piece lands; last slab: the
        output matmuls overlap the masking)."""
        for k in range(MPER):
            mc = slab * MPER + k
            if split:
                for j in range(4):
                    q0, q1 = j * 512, (j + 1) * 512
                    nc.vector.tensor_scalar(
                        v2[:, k * N + q0:k * N + q1], px.c_b[:, q0:q1],
                        px.edst[:, mc:mc + 1], px.edst02[:, mc:mc + 1],
                        OP.mult, OP.max,
                    )
            else:
                nc.vector.tensor_scalar(
                    v2[:, k * N:(k + 1) * N], px.c_b[:],
                    px.edst[:, mc:mc + 1], px.edst02[:, mc:mc + 1],
                    OP.mult, OP.max,
                )
            us[mc] = (u2, k * N)
        if split and eng is nc.vector:
            for j in range(4):
                nc.vector.tensor_tensor(
                    u2.rearrange("p (k q) -> p k q", q=N)[:, :, j * 512:(j + 1) * 512],
                    v2.rearrange("p (k q) -> p k q", q=N)[:, :, j * 512:(j + 1) * 512],
                    adj_slabs[slab].rearrange("p (k q) -> p k q", q=N)[:, :, j * 512:(j + 1) * 512],
                    OP.mult,
                )
        else:
            eng.tensor_tensor(u2[:], v2[:], adj_slabs[slab][:], OP.mult)

    gp = GP_SLAB if 0 <= GP_SLAB < NADJ else None
    rest = [s for s in range(NADJ) if s != gp]
    if gp is not None:
        # offloaded slab: scores first on DVE, mask on Pool (idle after the
        # adj dispatches drain)
        v2g = epool.tile([P, MPER * N], BF16, tag="v2gp", bufs=2,
                         name=f"v2g_{pair}")
        u2g = epool.tile([P, MPER * N], BF16, tag="u2gp", bufs=2,
                         name=f"u2g_{pair}")
        emit_slab(gp, v2g, u2g, split=(pair == 0), eng=nc.gpsimd)
    for i, slab in enumerate(rest):
        v2 = epool.tile([P, MPER * N], BF16, tag="v2", bufs=2,
                        name=f"v2_{pair}_{slab}")
        u2 = epool.tile([P, MPER * N], BF16, tag="u2",
                        bufs=3 if gp is not None else 5,
                        name=f"u2_{pair}_{slab}")
        split = (pair == 0 and gp is None and i == 0) or i == len(rest) - 1
        emit_slab(slab, v2, u2, split, nc.vector)

    first_mc = rest[0] * MPER
    last_slab = rest[-1]

    def emit_mm(mc, j):
        u_tile, uoff = us[mc]
        nc.tensor.matmul(
            psum_out[:, j * 512:(j + 1) * 512],
            px.hp_big[:, mc * W:(mc + 1) * W],
            u_tile[:, uoff + j * 512:uoff + (j + 1) * 512],
            start=(mc == first_mc),
            stop=(mc == last_slab * MPER + MPER - 1),
            skip_group_check=True,
        )

    for slab in rest[:-1]:
        for k in range(MPER):
            for j in range(4):
                emit_mm(slab * MPER + k, j)
    if gp is not None:
        # the Pool-produced chunks: ready long ago, emitted just before the
        # final slab so they never gate the accumulation
        for k in range(MPER):
            for j in range(4):
                emit_mm(gp * MPER + k, j)
    # last slab: j-major so each piece's matmuls hide behind the next
    # piece's mask op instead of queueing behind the final one
    for j in range(4):
        for k in range(MPER):
            emit_mm(last_slab * MPER + k, j)
    return psum_out


def _emit_b4(nc, cpool, pspool, ident_sb, pair, psum_out, out_d):
    """transpose back, normalize, store.  Pipelined by 512-col q pieces: the
    PSUM->SBUF copy (needed because PE can't read PSUM) runs per piece on
    ACT; the per-chunk reciprocal and scale read the transposed PSUM tile
    directly, so nothing round-trips through SBUF.

    pair 0's B4 overlaps pair 1's DVE-bound B3: its recips ride gpsimd and
    scales ride ACT so the DVE mask stream is never blocked.  pair 1's B4 is
    the kernel tail where the DVE is idle: recips + scales ride the DVE."""
    outT_sb = cpool.tile([W, N], F32, tag="outT", bufs=1, name=f"outT{pair}")
    outR = (cpool.tile([P, NCH * W], F32, tag="outR", bufs=1, name=f"outR{pair}")
            if pair == 0 else None)
    rec = cpool.tile([P, NCH], F32, tag="rec", bufs=2, name=f"rec{pair}")
    out_sb = cpool.tile([P, NCH * O], F32, tag="out_sb", bufs=1, name=f"outsb{pair}")
    GRP = 4
    for qg in range(NCH // GRP):
        # pair 1's tail: the DVE is idle, so split the piece copies and the
        # normalization across ACT+DVE and Pool+DVE respectively
        if pair == 1 and qg % 2 == 1:
            nc.vector.tensor_copy(outT_sb[:, qg * 512:(qg + 1) * 512],
                                  psum_out[:, qg * 512:(qg + 1) * 512])
        else:
            nc.scalar.copy(outT_sb[:, qg * 512:(qg + 1) * 512],
                           psum_out[:, qg * 512:(qg + 1) * 512])
        for qc in range(qg * GRP, (qg + 1) * GRP):
            psum_t = pspool.tile(
                [P, W], F32, tag="work", bufs=3, name=f"pst{pair}_{qc}"
            )
            nc.tensor.transpose(
                psum_t[:], outT_sb[:, qc * P:(qc + 1) * P], ident_sb[:W, :W]
            )
            # normalize by the ones-column rowsum.  Only the DVE can
            # reciprocal; pair 1 (kernel tail, DVE idle) runs recip+mul per
            # chunk straight from PSUM, pair 0 stages through SBUF on ACT
            # with one batched DVE recip per group so the DVE mask stream
            # is barely touched.
            if pair == 1:
                nc.vector.reciprocal(rec[:, qc:qc + 1], psum_t[:, O:O + 1])
                nc.vector.tensor_scalar_mul(
                    out_sb[:, qc * O:(qc + 1) * O], psum_t[:, 0:O],
                    rec[:, qc:qc + 1],
                )
            else:
                nc.scalar.copy(outR[:, qc * W:(qc + 1) * W], psum_t[:])
        if pair == 0:
            den = outR.rearrange("p (c k) -> p c k", k=W)[:, qg * GRP:(qg + 1) * GRP, O:O + 1]
            nc.vector.reciprocal(
                rec.rearrange("p (c k) -> p c k", k=1)[:, qg * GRP:(qg + 1) * GRP, :], den
            )
            for qc in range(qg * GRP, (qg + 1) * GRP):
                nc.scalar.activation(
                    out_sb[:, qc * O:(qc + 1) * O], outR[:, qc * W:qc * W + O],
                    AF.Copy, scale=rec[:, qc:qc + 1],
                )
        nc.sync.dma_start(
            out_d[pair].rearrange("(c p) o -> p c o", p=P)[:, qg * GRP:(qg + 1) * GRP, :],
            out_sb.rearrange("p (c k) -> p c k", k=O)[:, qg * GRP:(qg + 1) * GRP, :],
        )


def build_program(reps=1, loop_trip=None):
    nc = bacc.Bacc(
        "TRN2",
        target_bir_lowering=False,
        debug=False,
        enable_asserts=True,
        num_devices=1,
    )
    ht_d = nc.dram_tensor("ht", [F, N], F32, kind="ExternalInput").ap()
    adjt_d = nc.dram_tensor("adjt", [N, N], U8, kind="ExternalInput").ap()
    w_d = nc.dram_tensor("w", [2, F, O], F32, kind="ExternalInput").ap()
    a4_d = nc.dram_tensor("a4", [P, 8], F32, kind="ExternalInput").ap()
    biasb_d = nc.dram_tensor("biasb", [P, O], F32, kind="ExternalInput").ap()
    ident_d = nc.dram_tensor("ident", [P, P], F32, kind="ExternalInput").ap()
    sel_d = nc.dram_tensor("sel", [NCH, N], F32, kind="ExternalInput").ap()
    out_d = nc.dram_tensor("out", [2, N, O], F32, kind="ExternalOutput").ap()

    with tile.TileContext(nc) as tc, ExitStack() as ctx:
        consts_pool = ctx.enter_context(tc.tile_pool(name="consts", bufs=1))
        hpool = ctx.enter_context(tc.tile_pool(name="hpool", bufs=1))
        cpool = ctx.enter_context(tc.tile_pool(name="cpool", bufs=1))
        epool = ctx.enter_context(tc.tile_pool(name="epool", bufs=1))
        pspool = ctx.enter_context(tc.tile_pool(name="psum", bufs=1, space="PSUM"))

        # ACT func table preload: a dummy activation depending only on a
        # memset pulls the 1283ns LoadActFuncSet off the first tanh's
        # critical path.
        nbias_sb = consts_pool.tile([P, 1], F32, tag="nbias")
        nc.vector.memset(nbias_sb[:], U_SCALE_BIAS)
        actwarm = consts_pool.tile([1, 1], F32, tag="actwarm")
        nc.scalar.activation(actwarm[:], nbias_sb[0:1, 0:1], AF.Tanh)

        # --- priority DMAs, spread across three HWDGE queues (each queue
        # dispatches serially at ~625ns/DMA): w + h fc0 on sync, a4/sel +
        # h fc1 on scalar, ident/bias on vector.
        w_sb = consts_pool.tile([P, 2 * 2 * O], F32, tag="w")
        nc.sync.dma_start(
            w_sb.rearrange("k (h c o) -> k h c o", h=2, c=2),
            w_d.rearrange("h (c k) o -> k h c o", k=P),
        )
        a4_sb = consts_pool.tile([P, 8], F32, tag="a4")
        nc.scalar.dma_start(a4_sb[:], a4_d[:])
        bias_sb = consts_pool.tile([P, O], F32, tag="bias")
        nc.gpsimd.dma_start(bias_sb[:], biasb_d[:])

        hT = [
            hpool.tile([P, N], F32, tag=f"hT{fc}", name=f"hT{fc}")
            for fc in range(2)
        ]
        h_queues = (nc.sync, nc.scalar)
        h_dmas = [None] * 8
        ident_sb = consts_pool.tile([P, P], F32, tag="ident")
        for piece in range(4):
            for fc in range(2):
                h_dmas[2 * piece + fc] = h_queues[fc].dma_start(
                    hT[fc][:, piece * 512:(piece + 1) * 512],
                    ht_d[fc * P:(fc + 1) * P, piece * 512:(piece + 1) * 512],
                )
            if piece == 1:
                # ident is first needed by the g0 transpose (~tanh(0) time);
                # don't let it delay the first two h pieces.
                nc.scalar.dma_start(ident_sb[:], ident_d[:])
        sel_sb = consts_pool.tile([NCH, N], F32, tag="sel")
        nc.gpsimd.dma_start(sel_sb[:], sel_d[:])

        if loop_trip is not None:
            from concourse.engine_type import EngineType
            _loop_cm = tc.For_i(
                0, loop_trip, 1,
                hint_engines=(EngineType.PE, EngineType.DVE,
                              EngineType.Activation, EngineType.SP,
                              EngineType.Pool),
            )
            _loop_cm.__enter__()
        for rep in range(reps):
          # adjacency: NADJ independent slabs, fp16 via cast DMA (cast DMAs
          # must initiate from gpsimd)
          adj_slabs = [
              hpool.tile([P, MPER * N], BF16, tag=f"adj{s}", name=f"adj{s}")
              for s in range(NADJ)
          ]
          adjt_r = adjt_d.rearrange("(c p) q -> p c q", p=P)  # [128, 16, 2048]
          for s in range(NADJ):
              adj_dma = nc.gpsimd.dma_start(
                  adj_slabs[s].rearrange("p (c q) -> p c q", q=N),
                  adjt_r[:, s * MPER:(s + 1) * MPER, :],
              )
              # don't compete with the latency-critical h loads for DMA
              # bandwidth; the first B3 use of adj is ~10us in.
              add_dep_helper(adj_dma.ins, h_dmas[-1].ins,
                             reason="delay adj behind h")

          consts = (ident_sb, sel_sb, w_sb, a4_sb, bias_sb, nbias_sb)
          px = [_emit_b1b2(nc, cpool, pspool, consts, hT, pair)
                for pair in range(2)]
          for pair in range(2):
              psum_out = _emit_b3(nc, epool, pspool, adj_slabs, pair, px[pair])
              _emit_b4(nc, cpool, pspool, ident_sb, pair, psum_out, out_d)

        if loop_trip is not None:
            _loop_cm.__exit__(None, None, None)

    nc.compile()
    return nc


_CACHED = {}


def _get_program(zero_bias=True):
    global ZERO_BIAS
    key = ("nc", zero_bias)
    if key not in _CACHED:
        ZERO_BIAS = zero_bias
        _CACHED[key] = build_program()
    return _CACHED[key]


def make_in_maps(h, adj, w, a_src, a_dst, bias):
    h = np.ascontiguousarray(np.asarray(h, dtype=np.float32))
    adj = np.asarray(adj)
    w = np.asarray(w, dtype=np.float32)
    a_src = np.asarray(a_src, dtype=np.float32).reshape(4, O)
    a_dst = np.asarray(a_dst, dtype=np.float32).reshape(4, O)
    bias = np.asarray(bias, dtype=np.float32).reshape(O)

    adjT = np.ascontiguousarray(adj.transpose(0, 2, 1)).astype(np.uint8)
    biasb = np.ascontiguousarray(np.broadcast_to(bias, (P, O)))
    ident = np.eye(P, dtype=np.float32)
    sel = np.kron(np.eye(NCH, dtype=np.float32), np.ones((1, P), np.float32))

    in_maps = []
    for c in range(N_CORES):
        b = c // 2
        hs = [2 * (c % 2), 2 * (c % 2) + 1]
        # a4[:, 4*pair + (0..3)]: rows 0-63 (even chunk o's) carry
        # (a_src, a_dst, 0, 0); rows 64-127 (odd chunk o's) (0, 0, a_src,
        # a_dst) for that pair's head.
        a4 = np.zeros((P, 8), np.float32)
        for pair, hd in enumerate(hs):
            a4[0:O, 4 * pair + 0] = a_src[hd]
            a4[0:O, 4 * pair + 1] = a_dst[hd]
            a4[O:P, 4 * pair + 2] = a_src[hd]
            a4[O:P, 4 * pair + 3] = a_dst[hd]
        in_maps.append({
            "ht": np.ascontiguousarray(h[b].T),
            "adjt": adjT[b],
            "w": np.ascontiguousarray(w[hs]),
            "a4": a4,
            "biasb": biasb,
            "ident": ident,
            "sel": sel,
        })
    return in_maps


def assemble(results):
    out = np.empty((4, 4, N, O), dtype=np.float32)
    for c in range(N_CORES):
        b = c // 2
        for i, hd in enumerate((2 * (c % 2), 2 * (c % 2) + 1)):
            out[b, hd] = results[c]["out"][i]
    return out


def kernel(h, adj, w, a_src, a_dst, bias):
    nc = _get_program(zero_bias=not np.any(np.asarray(bias)))
    in_maps = make_in_maps(h, adj, w, a_src, a_dst, bias)
    res = run_bass_kernel_spmd(nc, in_maps, core_ids=list(range(N_CORES)))
    return assemble(res.results)


# revision 39
# speedup vs baseline: 1.3211x; 1.1367x over previous
"""Batch multi-head graph attention (GAT) kernel for 8 Trainium2 NeuronCores.

Reference computation (per batch b, head g):
    hp   = h[b] @ w[g]                        # [N, O]
    t    = tanh(hp)
    src  = t @ a_src[g];  dst = t @ a_dst[g]  # [N]
    s    = leaky_relu(src[q] + dst[m], 0.2)   # [N(q), N(m)]
    e    = exp(s) masked by adj[b][q, m]
    out  = (e @ hp) / rowsum(e) + bias

Device strategy (per core; core c -> b = c//2, heads = (2*(c%2), 2*(c%2)+1)):
  * scores are built TRANSPOSED: tiles [128 m(keys), 2048 q(queries)] so the
    output matmul out^T[o, q] = sum_m hp[m, o] * e[m, q] streams on PE with
    the contraction dim on partitions.  The adjacency mask is pre-transposed
    on the host and streamed in via a dtype-casting DMA (u8 -> bf16).
  * exp factorization with query-scale cancellation:
        exp(leaky(s)) = max(exp(s), exp(0.2 s))
                      = exp(0.2 src)[q] * max(exp(0.8 src)[q] * exp(dst)[m],
                                              exp(0.2 dst)[m])
    Both the output numerator and the softmax denominator are accumulated by
    the same matmul, so the common per-query factor exp(0.2 src)[q] cancels
    in the final division and is never computed.  Per m-chunk the scores are
    ONE dual-op tensor_scalar (DVE 4x perf mode):
        u = (c_b * P[m]) max Q[m]      c_b = exp(0.8 src) broadcast tile
    plus one mask multiply per adjacency slab.
  * B2 rides PE/ACT instead of DVE: per 2-chunk group, PE transposes the
    tanh block [128 m, 128 (c,o)] -> [128 (c,o), 128 m] and a 4-column
    matvec against (a_src|a_dst) yields sd[m, (src,dst) x 2 chunks] directly
    partition-oriented.  exp(dst)/exp(0.2 dst) come off sd with strided ACT
    exps; exp(0.8 src) is broadcast via the sel-matmul trick.  The DVE does
    nothing in B1/B2 except the hp PSUM->SBUF copies.
  * softmax denominator rides as a ones-column in the matmul lhsT; bias is
    folded into hp (softmax weights sum to exactly 1).  Final transpose back
    to [q, o] via PE transpose; normalization runs on the scalar engine.
  * PSUM banks (8): B1/B2/B4 transient tiles share a 3-bank rotation
    ("work"), sd gets 1 persistent bank, the output accumulator takes 4.
"""

import math
import os
import sys

for _p in ("/opt/trn_rl_repo",):
    if _p not in sys.path and os.path.isdir(_p):
        sys.path.insert(0, _p)

from contextlib import ExitStack

import numpy as np

import concourse.bass as bass
import concourse.tile as tile
from concourse import bacc, mybir
from concourse.bass_utils import run_bass_kernel_spmd
from concourse.tile_rust import add_dep_helper

F32 = mybir.dt.float32
BF16 = mybir.dt.float16  # fp16: more mantissa + possible DVE fast-path
U8 = mybir.dt.uint8
F8 = mybir.dt.float8e4
AF = mybir.ActivationFunctionType
OP = mybir.AluOpType
AX = mybir.AxisListType

N = 2048          # nodes
F = 256           # input features
O = 64            # output features
W = O + 1         # hp columns + ones column
P = 128           # partitions
NCH = N // P      # 16 m-chunks per pair
NGR = NCH // 2    # 2-chunk B1 groups
NEG_SLOPE = 0.2
N_CORES = 8
NADJ = 8          # adj arrives as NADJ independent slabs for overlap
MPER = NCH // NADJ

# masked scores written as fp8e4.  REJECTED on HW: 150.6us vs 104.2us -- the
# 1-byte output drops the mask tensor_tensor off the DVE's 2-byte fast path
# (HW rel err 1.55e-2 would have passed the 2e-2 gate, but the perf loss is
# decisive).  Keep False.
U_FP8 = os.environ.get("KERNEL_U_FP8", "0") == "1"
# offload this slab index of each pair's mask to the Pool engine (fp16 mult,
# proven op): its tensor_scalars emit at the head of the pair's B3 so Pool
# can start right after the adj dispatches drain; its matmuls emit last so a
# slow Pool never blocks the accumulation.  -1 disables.
GP_SLAB = int(os.environ.get("KERNEL_GP_SLAB", "-1"))
U_SCALE_BIAS = -math.log(16.0)  # exp bias: scales cb and Q by 1/16
# when the bias input is all-zero (it is for this problem), hp needs no add;
# set per-build from the actual bias values in kernel()
ZERO_BIAS = True


class PairCtx:
    pass


def _emit_b1b2(nc, cpool, pspool, consts, hT, pair):
    """projections + score ingredients for one head (pair index).

    Per 2-chunk group: hp matmuls into a [128, 128] PSUM group tile, one
    copy to hp_big (bf16, strided over the ones column), one ACT tanh, a PE
    transpose of the tanh block, an ACT copy of the transpose to SBUF, and a
    4-col PE matvec accumulating sd[m, (src,dst)] per chunk.

    pair 0's small copies ride the DVE (idle during B1); pair 1's ride ACT
    so they never sit in the DVE queue ahead of pair 0's mask stream."""
    ident_sb, sel_sb, w_sb, a4_sb, bias_sb, nbias_sb = consts

    def copy(dst, src):
        if pair == 0:
            nc.vector.tensor_copy(dst, src)
        else:
            nc.scalar.copy(dst, src)
    px = PairCtx()
    px.hp_big = cpool.tile([P, NCH * W], BF16, tag="hp_big", bufs=2, name=f"hp{pair}")
    px.t_cat = cpool.tile([P, NCH * O], F32, tag="t_cat", bufs=2, name=f"tcat{pair}")

    sd_ps = pspool.tile([P, 4 * NGR], F32, tag="sd", bufs=1, name=f"sd{pair}")

    def emit_group_head(g):
        psum_hp = pspool.tile([P, 2 * O], F32, tag="work", bufs=3,
                              name=f"pshp{pair}_g{g}")
        for k in range(2):
            mc = 2 * g + k
            for fc in range(2):
                nc.tensor.matmul(
                    psum_hp[:, k * O:(k + 1) * O],
                    hT[fc][:, mc * P:(mc + 1) * P],
                    w_sb[:, (2 * pair + fc) * O:(2 * pair + fc + 1) * O],
                    start=(fc == 0),
                    stop=(fc == 1),
                )
        if ZERO_BIAS:
            copy(
                px.hp_big.rearrange("p (c k) -> p c k", k=W)[:, 2 * g:2 * g + 2, 0:O],
                psum_hp.rearrange("p (c k) -> p c k", k=O),
            )
        else:
            for k in range(2):
                mc = 2 * g + k
                nc.vector.tensor_tensor(
                    px.hp_big[:, mc * W:mc * W + O],
                    psum_hp[:, k * O:(k + 1) * O], bias_sb[:], OP.add,
                )
        nc.scalar.activation(
            px.t_cat[:, 2 * g * O:(2 * g + 2) * O], psum_hp[:], AF.Tanh
        )
        return psum_hp

    def emit_group_tail(g):
        # transpose the tanh block: [128 m, 128 (c,o)] -> [128 (c,o), 128 m]
        tT2_ps = pspool.tile([P, P], F32, tag="work", bufs=3,
                             name=f"tT2ps{pair}_{g}")
        nc.tensor.transpose(
            tT2_ps[:], px.t_cat[:, 2 * g * O:(2 * g + 2) * O], ident_sb[:]
        )
        tT2_sb = cpool.tile([P, P], F32, tag="tT2", bufs=3, name=f"tT2{pair}_{g}")
        copy(tT2_sb[:], tT2_ps[:])
        # matvec: out[m, (src_e, dst_e, src_o, dst_o)] for the group's chunks
        nc.tensor.matmul(
            sd_ps[:, 4 * g:4 * g + 4],
            tT2_sb[:],
            a4_sb[:, 4 * pair:4 * pair + 4],
            start=True, stop=True,
        )

    # ---- B2, pipelined per 4-chunk piece inside the B1 loop ----
    # c_b piece j (q chunks 4j..4j+3) only needs src of those chunks, i.e.
    # sd groups 2j, 2j+1 — so the exp vectors and the broadcast build
    # stream during B1 and the first B3 tensor_scalar (which reads c_b
    # per 512-col piece) starts ~5us earlier.
    sd_sb = cpool.tile([P, 4 * NGR], F32, tag="sd_sb", bufs=2, name=f"sdsb{pair}")
    sd3 = sd_sb.rearrange("p (c k) -> p c k", k=2)
    px.edst = cpool.tile([P, NCH], F32, tag="edst", bufs=2, name=f"edst{pair}")
    px.edst02 = cpool.tile([P, NCH], F32, tag="edst02", bufs=2, name=f"edst02{pair}")
    # piece j's 4 src rows live at partitions 0-3, free offset j*128 (PE
    # operands must start at base partition 0/32/64)
    srowT = cpool.tile([4, 4 * P], F32, tag="srowT", bufs=2, name=f"srowT{pair}")
    px.c_b = cpool.tile([P, N], BF16, tag="c_b", bufs=2, name=f"cb{pair}")

    def emit_b2_piece_a(j):
        # exp vectors + src-row transpose for q-chunks 4j..4j+3
        c0 = 4 * j
        copy(sd_sb[:, 8 * j:8 * j + 8], sd_ps[:, 8 * j:8 * j + 8])
        nc.scalar.activation(px.edst[:, c0:c0 + 4], sd3[:, c0:c0 + 4, 1],
                             AF.Exp)
        nc.scalar.activation(px.edst02[:, c0:c0 + 4], sd3[:, c0:c0 + 4, 1],
                             AF.Exp, scale=NEG_SLOPE,
                             bias=nbias_sb[:] if U_FP8 else 0.0)
        ps_srcT = pspool.tile([4, P], F32, tag="work", bufs=3,
                              name=f"srcT{pair}_{j}")
        nc.tensor.transpose(ps_srcT[:], sd3[:, c0:c0 + 4, 0], ident_sb[:])
        copy(srowT[:, j * P:(j + 1) * P], ps_srcT[:])

    def emit_b2_piece_b(j):
        # broadcast build for the piece
        ps_bc = pspool.tile([P, 512], F32, tag="work", bufs=3,
                            name=f"psbc{pair}_{j}")
        for c4 in range(4):
            c = 4 * j + c4
            nc.tensor.matmul(
                ps_bc[:, c4 * P:(c4 + 1) * P],
                sel_sb[:, c * P:(c + 1) * P],
                srowT[:, j * P:(j + 1) * P], start=True, stop=True,
            )
        nc.scalar.activation(
            px.c_b[:, j * 512:(j + 1) * 512], ps_bc[:], AF.Exp,
            scale=1.0 - NEG_SLOPE,
            bias=nbias_sb[:] if U_FP8 else 0.0,
        )

    # software-pipelined emission: group g's PE tail follows group g+1's
    # matmuls so the in-order PE queue never stalls on ACT's tanh.  B2
    # piece j (needs sd groups 2j, 2j+1) emits its DVE/transpose phase one
    # group later and its broadcast matmuls another group later, so each
    # cross-engine round-trip hides behind a B1 group's PE work.
    for g in range(NGR):
        emit_group_head(g)
        if g >= 1:
            emit_group_tail(g - 1)
        if g >= 2 and g % 2 == 0:
            emit_b2_piece_a(g // 2 - 1)
        if g >= 3 and g % 2 == 1:
            emit_b2_piece_b((g - 1) // 2 - 1)
    emit_group_tail(NGR - 1)
    emit_b2_piece_a(NGR // 2 - 1)
    emit_b2_piece_b(NGR // 2 - 1)

    ones_cols = px.hp_big.rearrange("p (c k) -> p c k", k=W)[:, :, O:O + 1]
    (nc.vector if pair == 0 else nc.gpsimd).memset(ones_cols, 1.0)
    return px


def _emit_b3(nc, epool, pspool, adj_slabs, pair, px):
    """scores + output matmul accumulation over m-chunks."""
    psum_out = pspool.tile([W, N], F32, tag="big", bufs=1, name=f"psout{pair}")
    us = {}  # mc -> (tile, element offset of the chunk within the tile)

    def emit_slab(slab, v2, u2, split, eng):
        """one mask tensor_tensor per adjacency slab (MPER chunks): halves
        the per-op overhead + semaphore traffic on the DVE's critical
        stream.  split=True splits the score ops per 512-col q piece (first
        slab: start as soon as c_b's first piece lands; last slab: the
        output matmuls overlap the masking)."""
        for k in range(MPER):
            mc = slab * MPER + k
            if split:
                for j in range(4):
                    q0, q1 = j * 512, (j + 1) * 512
                    nc.vector.tensor_scalar(
                        v2[:, k * N + q0:k * N + q1], px.c_b[:, q0:q1],
                        px.edst[:, mc:mc + 1], px.edst02[:, mc:mc + 1],
                        OP.mult, OP.max,
                    )
            else:
                nc.vector.tensor_scalar(
                    v2[:, k * N:(k + 1) * N], px.c_b[:],
                    px.edst[:, mc:mc + 1], px.edst02[:, mc:mc + 1],
                    OP.mult, OP.max,
                )
            us[mc] = (u2, k * N)
        if split and eng is nc.vector:
            for j in range(4):
                nc.vector.tensor_tensor(
                    u2.rearrange("p (k q) -> p k q", q=N)[:, :, j * 512:(j + 1) * 512],
                    v2.rearrange("p (k q) -> p k q", q=N)[:, :, j * 512:(j + 1) * 512],
                    adj_slabs[slab].rearrange("p (k q) -> p k q", q=N)[:, :, j * 512:(j + 1) * 512],
                    OP.mult,
                )
        else:
            eng.tensor_tensor(u2[:], v2[:], adj_slabs[slab][:], OP.mult)

    gp = GP_SLAB if 0 <= GP_SLAB < NADJ else None
    rest = [s for s in range(NADJ) if s != gp]
    if gp is not None:
        # offloaded slab: scores first on DVE, mask on Pool (idle after the
        # adj dispatches drain)
        v2g = epool.tile([P, MPER * N], BF16, tag="v2gp", bufs=2,
                         name=f"v2g_{pair}")
        u2g = epool.tile([P, MPER * N], BF16, tag="u2gp", bufs=2,
                         name=f"u2g_{pair}")
        emit_slab(gp, v2g, u2g, split=(pair == 0), eng=nc.gpsimd)
    for i, slab in enumerate(rest):
        v2 = epool.tile([P, MPER * N], BF16, tag="v2", bufs=2,
                        name=f"v2_{pair}_{slab}")
        u2 = epool.tile([P, MPER * N], BF16, tag="u2",
                        bufs=3 if gp is not None else 5,
                        name=f"u2_{pair}_{slab}")
        # split slab 0 of pair 0 (start as soon as c_b piece 0 lands) and
        # the final slab of pair 1 (the kernel tail); pair 0's last slab
        # flows straight into pair 1's DVE stream, so a split there is
        # pure per-op overhead.
        split = (pair == 0 and gp is None and i == 0) or (
            pair == 1 and i == len(rest) - 1)
        emit_slab(slab, v2, u2, split, nc.vector)

    first_mc = rest[0] * MPER
    last_slab = rest[-1]

    def emit_mm(mc, j):
        u_tile, uoff = us[mc]
        nc.tensor.matmul(
            psum_out[:, j * 512:(j + 1) * 512],
            px.hp_big[:, mc * W:(mc + 1) * W],
            u_tile[:, uoff + j * 512:uoff + (j + 1) * 512],
            start=(mc == first_mc),
            stop=(mc == last_slab * MPER + MPER - 1),
            skip_group_check=True,
        )

    for slab in rest[:-1]:
        for k in range(MPER):
            for j in range(4):
                emit_mm(slab * MPER + k, j)
    if gp is not None:
        # the Pool-produced chunks: ready long ago, emitted just before the
        # final slab so they never gate the accumulation
        for k in range(MPER):
            for j in range(4):
                emit_mm(gp * MPER + k, j)
    # last slab: j-major so each piece's matmuls hide behind the next
    # piece's mask op instead of queueing behind the final one
    for j in range(4):
        for k in range(MPER):
            emit_mm(last_slab * MPER + k, j)
    return psum_out


def _emit_b4(nc, cpool, pspool, ident_sb, pair, psum_out, out_d):
    """transpose back, normalize, store.  Pipelined by 512-col q pieces: the
    PSUM->SBUF copy (needed because PE can't read PSUM) runs per piece on
    ACT; the per-chunk reciprocal and scale read the transposed PSUM tile
    directly, so nothing round-trips through SBUF.

    pair 0's B4 overlaps pair 1's DVE-bound B3: its recips ride gpsimd and
    scales ride ACT so the DVE mask stream is never blocked.  pair 1's B4 is
    the kernel tail where the DVE is idle: recips + scales ride the DVE."""
    outT_sb = cpool.tile([W, N], F32, tag="outT", bufs=1, name=f"outT{pair}")
    outR = (cpool.tile([P, NCH * W], F32, tag="outR", bufs=1, name=f"outR{pair}")
            if pair == 0 else None)
    rec = cpool.tile([P, NCH], F32, tag="rec", bufs=2, name=f"rec{pair}")
    out_sb = cpool.tile([P, NCH * O], F32, tag="out_sb", bufs=1, name=f"outsb{pair}")
    GRP = 4
    for qg in range(NCH // GRP):
        # pair 1's tail: the DVE is idle, so split the piece copies and the
        # normalization across ACT+DVE and Pool+DVE respectively
        if pair == 1 and qg % 2 == 1:
            nc.vector.tensor_copy(outT_sb[:, qg * 512:(qg + 1) * 512],
                                  psum_out[:, qg * 512:(qg + 1) * 512])
        else:
            nc.scalar.copy(outT_sb[:, qg * 512:(qg + 1) * 512],
                           psum_out[:, qg * 512:(qg + 1) * 512])
        for qc in range(qg * GRP, (qg + 1) * GRP):
            psum_t = pspool.tile(
                [P, W], F32, tag="work", bufs=3, name=f"pst{pair}_{qc}"
            )
            nc.tensor.transpose(
                psum_t[:], outT_sb[:, qc * P:(qc + 1) * P], ident_sb[:W, :W]
            )
            # normalize by the ones-column rowsum.  Only the DVE can
            # reciprocal; pair 1 (kernel tail, DVE idle) runs recip+mul per
            # chunk straight from PSUM, pair 0 stages through SBUF on ACT
            # with one batched DVE recip per group so the DVE mask stream
            # is barely touched.
            if pair == 1:
                nc.vector.reciprocal(rec[:, qc:qc + 1], psum_t[:, O:O + 1])
                nc.vector.tensor_scalar_mul(
                    out_sb[:, qc * O:(qc + 1) * O], psum_t[:, 0:O],
                    rec[:, qc:qc + 1],
                )
            else:
                nc.scalar.copy(outR[:, qc * W:(qc + 1) * W], psum_t[:])
        if pair == 0:
            den = outR.rearrange("p (c k) -> p c k", k=W)[:, qg * GRP:(qg + 1) * GRP, O:O + 1]
            nc.vector.reciprocal(
                rec.rearrange("p (c k) -> p c k", k=1)[:, qg * GRP:(qg + 1) * GRP, :], den
            )
            for qc in range(qg * GRP, (qg + 1) * GRP):
                nc.scalar.activation(
                    out_sb[:, qc * O:(qc + 1) * O], outR[:, qc * W:qc * W + O],
                    AF.Copy, scale=rec[:, qc:qc + 1],
                )
        nc.sync.dma_start(
            out_d[pair].rearrange("(c p) o -> p c o", p=P)[:, qg * GRP:(qg + 1) * GRP, :],
            out_sb.rearrange("p (c k) -> p c k", k=O)[:, qg * GRP:(qg + 1) * GRP, :],
        )


def build_program(reps=1, loop_trip=None):
    nc = bacc.Bacc(
        "TRN2",
        target_bir_lowering=False,
        debug=False,
        enable_asserts=True,
        num_devices=1,
    )
    ht_d = nc.dram_tensor("ht", [F, N], F32, kind="ExternalInput").ap()
    adjt_d = nc.dram_tensor("adjt", [N, N], U8, kind="ExternalInput").ap()
    w_d = nc.dram_tensor("w", [2, F, O], F32, kind="ExternalInput").ap()
    a4_d = nc.dram_tensor("a4", [P, 8], F32, kind="ExternalInput").ap()
    biasb_d = nc.dram_tensor("biasb", [P, O], F32, kind="ExternalInput").ap()
    ident_d = nc.dram_tensor("ident", [P, P], F32, kind="ExternalInput").ap()
    sel_d = nc.dram_tensor("sel", [4, N], F32, kind="ExternalInput").ap()
    out_d = nc.dram_tensor("out", [2, N, O], F32, kind="ExternalOutput").ap()

    with tile.TileContext(nc) as tc, ExitStack() as ctx:
        consts_pool = ctx.enter_context(tc.tile_pool(name="consts", bufs=1))
        hpool = ctx.enter_context(tc.tile_pool(name="hpool", bufs=1))
        cpool = ctx.enter_context(tc.tile_pool(name="cpool", bufs=1))
        epool = ctx.enter_context(tc.tile_pool(name="epool", bufs=1))
        pspool = ctx.enter_context(tc.tile_pool(name="psum", bufs=1, space="PSUM"))

        # ACT func table preload: a dummy activation depending only on a
        # memset pulls the 1283ns LoadActFuncSet off the first tanh's
        # critical path.
        nbias_sb = consts_pool.tile([P, 1], F32, tag="nbias")
        nc.vector.memset(nbias_sb[:], U_SCALE_BIAS)
        actwarm = consts_pool.tile([1, 1], F32, tag="actwarm")
        nc.scalar.activation(actwarm[:], nbias_sb[0:1, 0:1], AF.Tanh)

        # --- priority DMAs, spread across three HWDGE queues (each queue
        # dispatches serially at ~625ns/DMA): w + h fc0 on sync, a4/sel +
        # h fc1 on scalar, ident/bias on vector.
        w_sb = consts_pool.tile([P, 2 * 2 * O], F32, tag="w")
        nc.sync.dma_start(
            w_sb.rearrange("k (h c o) -> k h c o", h=2, c=2),
            w_d.rearrange("h (c k) o -> k h c o", k=P),
        )
        a4_sb = consts_pool.tile([P, 8], F32, tag="a4")
        nc.scalar.dma_start(a4_sb[:], a4_d[:])
        bias_sb = consts_pool.tile([P, O], F32, tag="bias")
        nc.gpsimd.dma_start(bias_sb[:], biasb_d[:])

        hT = [
            hpool.tile([P, N], F32, tag=f"hT{fc}", name=f"hT{fc}")
            for fc in range(2)
        ]
        h_queues = (nc.sync, nc.scalar)
        h_dmas = [None] * 8
        ident_sb = consts_pool.tile([P, P], F32, tag="ident")
        for piece in range(4):
            for fc in range(2):
                h_dmas[2 * piece + fc] = h_queues[fc].dma_start(
                    hT[fc][:, piece * 512:(piece + 1) * 512],
                    ht_d[fc * P:(fc + 1) * P, piece * 512:(piece + 1) * 512],
                )
            if piece == 1:
                # ident is first needed by the g0 transpose (~tanh(0) time);
                # don't let it delay the first two h pieces.
                nc.scalar.dma_start(ident_sb[:], ident_d[:])
        sel_sb = consts_pool.tile([4, N], F32, tag="sel")
        nc.gpsimd.dma_start(sel_sb[:], sel_d[:])

        if loop_trip is not None:
            from concourse.engine_type import EngineType
            _loop_cm = tc.For_i(
                0, loop_trip, 1,
                hint_engines=(EngineType.PE, EngineType.DVE,
                              EngineType.Activation, EngineType.SP,
                              EngineType.Pool),
            )
            _loop_cm.__enter__()
        for rep in range(reps):
          # adjacency: NADJ independent slabs, fp16 via cast DMA (cast DMAs
          # must initiate from gpsimd)
          adj_slabs = [
              hpool.tile([P, MPER * N], BF16, tag=f"adj{s}", name=f"adj{s}")
              for s in range(NADJ)
          ]
          adjt_r = adjt_d.rearrange("(c p) q -> p c q", p=P)  # [128, 16, 2048]
          for s in range(NADJ):
              adj_dma = nc.gpsimd.dma_start(
                  adj_slabs[s].rearrange("p (c q) -> p c q", q=N),
                  adjt_r[:, s * MPER:(s + 1) * MPER, :],
              )
              # don't compete with the latency-critical h loads for DMA
              # bandwidth; the first B3 use of adj is ~12us in.
              add_dep_helper(adj_dma.ins, h_dmas[-1].ins,
                             reason="delay adj behind h")

          consts = (ident_sb, sel_sb, w_sb, a4_sb, bias_sb, nbias_sb)
          px = [_emit_b1b2(nc, cpool, pspool, consts, hT, pair)
                for pair in range(2)]
          for pair in range(2):
              psum_out = _emit_b3(nc, epool, pspool, adj_slabs, pair, px[pair])
              _emit_b4(nc, cpool, pspool, ident_sb, pair, psum_out, out_d)

        if loop_trip is not None:
            _loop_cm.__exit__(None, None, None)

    nc.compile()
    return nc


_CACHED = {}


def _get_program(zero_bias=True):
    global ZERO_BIAS
    key = ("nc", zero_bias)
    if key not in _CACHED:
        ZERO_BIAS = zero_bias
        _CACHED[key] = build_program()
    return _CACHED[key]


def make_in_maps(h, adj, w, a_src, a_dst, bias):
    h = np.ascontiguousarray(np.asarray(h, dtype=np.float32))
    adj = np.asarray(adj)
    w = np.asarray(w, dtype=np.float32)
    a_src = np.asarray(a_src, dtype=np.float32).reshape(4, O)
    a_dst = np.asarray(a_dst, dtype=np.float32).reshape(4, O)
    bias = np.asarray(bias, dtype=np.float32).reshape(O)

    adjT = np.ascontiguousarray(adj.transpose(0, 2, 1)).astype(np.uint8)
    biasb = np.ascontiguousarray(np.broadcast_to(bias, (P, O)))
    ident = np.eye(P, dtype=np.float32)
    # sel[r, q] = 1 iff q's 128-chunk index == r mod 4: the one-hot selector
    # for the piecewise c_b broadcast (piece-invariant since row = chunk%4)
    sel = np.kron(np.tile(np.eye(4, dtype=np.float32), (1, NCH // 4)),
                  np.ones((1, P), np.float32))

    in_maps = []
    for c in range(N_CORES):
        b = c // 2
        hs = [2 * (c % 2), 2 * (c % 2) + 1]
        # a4[:, 4*pair + (0..3)]: rows 0-63 (even chunk o's) carry
        # (a_src, a_dst, 0, 0); rows 64-127 (odd chunk o's) (0, 0, a_src,
        # a_dst) for that pair's head.
        a4 = np.zeros((P, 8), np.float32)
        for pair, hd in enumerate(hs):
            a4[0:O, 4 * pair + 0] = a_src[hd]
            a4[0:O, 4 * pair + 1] = a_dst[hd]
            a4[O:P, 4 * pair + 2] = a_src[hd]
            a4[O:P, 4 * pair + 3] = a_dst[hd]
        in_maps.append({
            "ht": np.ascontiguousarray(h[b].T),
            "adjt": adjT[b],
            "w": np.ascontiguousarray(w[hs]),
            "a4": a4,
            "biasb": biasb,
            "ident": ident,
            "sel": sel,
        })
    return in_maps


def assemble(results):
    out = np.empty((4, 4, N, O), dtype=np.float32)
    for c in range(N_CORES):
        b = c // 2
        for i, hd in enumerate((2 * (c % 2), 2 * (c % 2) + 1)):
            out[b, hd] = results[c]["out"][i]
    return out


def kernel(h, adj, w, a_src, a_dst, bias):
    nc = _get_program(zero_bias=not np.any(np.asarray(bias)))
    in_maps = make_in_maps(h, adj, w, a_src, a_dst, bias)
    res = run_bass_kernel_spmd(nc, in_maps, core_ids=list(range(N_CORES)))
    return assemble(res.results)
